# revision 1
# baseline (speedup 1.0000x reference)
"""MCR2 loss kernel for 8 Trainium2 NeuronCores.

Data-parallel over the sample axis: each core streams its 75000-row shard
of Z once, building per-128-sample-tile one-hot-masked copies of Z with a
single fused DVE scalar_tensor_tensor op (M[p, j*32+f] = (j == label_p) *
Z[p, f]) and accumulating Z_tile^T @ M_tile into PSUM, which yields all 10
per-class Grams Gj = Z^T diag(Pi_j) Z.  G = sum_j Gj exactly (one-hot).
The tiny [10,32,32] partials are summed on the host and the 32x32 logdets
are evaluated there in float64.
"""

import os
import sys

sys.path.insert(0, "/opt/trn_rl_repo")

import numpy as np

import concourse.bacc as bacc
import concourse.bass as bass
import concourse.mybir as mybir
import concourse.tile as tile
from concourse.bass_utils import run_bass_kernel_spmd

N, D, C = 600000, 32, 10
EPS = 0.5
NCORES = 8
PER = N // NCORES            # 75000 rows per core
PAD = ((PER + 127) // 128) * 128   # 75008
NTILES = PAD // 128          # 586 tiles of 128 samples
T_FULL = 32                  # tiles per chunk
FULL_CHUNKS = NTILES // T_FULL      # 9
T_TAIL = NTILES - FULL_CHUNKS * T_FULL  # 10
MW = C * D                   # 320: masked block width

_cache = {}


def _build_program():
    nc = bacc.Bacc(None)
    z_dram = nc.dram_tensor("Z", [PAD, D], mybir.dt.float32, kind="ExternalInput")
    lab_dram = nc.dram_tensor("labels", [PAD], mybir.dt.int32, kind="ExternalInput")
    out_dram = nc.dram_tensor("grams", [128, MW], mybir.dt.float32, kind="ExternalOutput")

    # class-index constant, value j repeated D times: [128, 320] bf16
    iota_np = np.tile(np.repeat(np.arange(C), D)[None, :], (128, 1)).astype(
        np.dtype("bfloat16") if hasattr(np, "bfloat16") else np.float32
    )
    # ml_dtypes bfloat16 via mybir numpy mapping
    import ml_dtypes

    iota_np = np.tile(np.arange(C)[None, :], (128, 1)).astype(ml_dtypes.bfloat16)
    iota_dram = nc.inline_tensor(iota_np, name="iota_c")

    bf16 = mybir.dt.bfloat16
    f32 = mybir.dt.float32

    with tile.TileContext(nc) as tc:
        with (
            tc.tile_pool(name="zraw", bufs=2) as zraw_pool,
            tc.tile_pool(name="zin", bufs=2) as zin_pool,
            tc.tile_pool(name="lab", bufs=2) as lab_pool,
            tc.tile_pool(name="labf", bufs=2) as labf_pool,
            tc.tile_pool(name="mask", bufs=2) as m_pool,
            tc.tile_pool(name="mask10", bufs=2) as mk_pool,
            tc.tile_pool(name="const", bufs=1) as const_pool,
            tc.tile_pool(name="outp", bufs=1) as out_pool,
            tc.tile_pool(name="psum", bufs=1, space="PSUM") as psum_pool,
        ):
            iota_sb = const_pool.tile([128, C], bf16)
            nc.sync.dma_start(iota_sb[:], iota_dram[:])
            # Tiny DVE read of the const so the DVE engine's vector clock
            # observes the const DMA once, instead of the wait landing on a
            # later STT (walrus: "Too many sync wait commands").
            touch = const_pool.tile([128, 2], bf16)
            nc.vector.tensor_copy(touch[:], iota_sb[:, 0:2])

            acc = psum_pool.tile([128, MW], f32)

            z_full = z_dram[0 : FULL_CHUNKS * 128 * T_FULL, :].rearrange(
                "(c p t) d -> c p (t d)", p=128, t=T_FULL
            )
            lab_full = lab_dram[0 : FULL_CHUNKS * 128 * T_FULL].rearrange(
                "(c p t) -> c p t", p=128, t=T_FULL
            )
            z_tail = z_dram[FULL_CHUNKS * 128 * T_FULL :, :].rearrange(
                "(p t) d -> p (t d)", p=128, t=T_TAIL
            )
            lab_tail = lab_dram[FULL_CHUNKS * 128 * T_FULL :].rearrange(
                "(p t) -> p t", p=128, t=T_TAIL
            )

            gtile = 0
            for c in range(FULL_CHUNKS + 1):
                tchunk = T_FULL if c < FULL_CHUNKS else T_TAIL
                z_raw = zraw_pool.tile([128, T_FULL * D], f32, tag="zr")
                z_sb = zin_pool.tile([128, T_FULL * D], bf16, tag="z")
                lab_sb = lab_pool.tile([128, T_FULL], mybir.dt.int32, tag="l")
                labf_sb = labf_pool.tile([128, T_FULL], bf16, tag="lf")
                if c < FULL_CHUNKS:
                    nc.sync.dma_start(z_raw[:, : tchunk * D], z_full[c])
                    nc.sync.dma_start(lab_sb[:, :tchunk], lab_full[c])
                else:
                    nc.sync.dma_start(z_raw[:, : tchunk * D], z_tail[:])
                    nc.sync.dma_start(lab_sb[:, :tchunk], lab_tail[:])
                nc.vector.tensor_copy(labf_sb[:, :tchunk], lab_sb[:, :tchunk])
                # fp32 -> bf16 cast on the otherwise-idle Scalar engine; also
                # the single sync point between the Z DMA and downstream readers.
                nc.scalar.mul(z_sb[:, : tchunk * D], z_raw[:, : tchunk * D], 1.0)

                # one-hot mask for the whole chunk: [128, t, j]
                mk_sb = mk_pool.tile([128, T_FULL * C], bf16, tag="mk")
                nc.vector.tensor_tensor(
                    out=mk_sb[:, : tchunk * C].rearrange("p (t j) -> p t j", j=C),
                    in0=labf_sb[:, :tchunk].unsqueeze(2).broadcast_to(
                        [128, tchunk, C]
                    ),
                    in1=iota_sb[:].unsqueeze(1).broadcast_to([128, tchunk, C]),
                    op=mybir.AluOpType.is_equal,
                )
                # masked copies for the whole chunk in one wide multiply:
                # M[p, t, j, f] = mask[p, t, j] * Z[p, t, f]
                m_sb = m_pool.tile([128, T_FULL * MW], bf16, tag="m")
                for eng, lo, hi in ((nc.vector, 0, tchunk),):
                    nt = hi - lo
                    eng.tensor_tensor(
                        out=m_sb[:, lo * MW : hi * MW].rearrange(
                            "p (t j f) -> p t j f", j=C, f=D
                        ),
                        in0=mk_sb[:, lo * C : hi * C]
                        .rearrange("p (t j) -> p t j", j=C)
                        .unsqueeze(3)
                        .broadcast_to([128, nt, C, D]),
                        in1=z_sb[:, lo * D : hi * D]
                        .rearrange("p (t f) -> p t f", f=D)
                        .unsqueeze(2)
                        .broadcast_to([128, nt, C, D]),
                        op=mybir.AluOpType.mult,
                    )
                for t in range(tchunk):
                    grp = gtile % 4
                    nc.tensor.matmul(
                        acc[grp * D : (grp + 1) * D, :],
                        z_sb[:, t * D : (t + 1) * D],
                        m_sb[:, t * MW : (t + 1) * MW],
                        start=(gtile < 4),
                        stop=(gtile >= NTILES - 4),
                        tile_position=(0, grp * D),
                    )
                    gtile += 1

            out_sb = out_pool.tile([128, MW], f32)
            nc.vector.tensor_copy(out_sb[:], acc[:])
            nc.sync.dma_start(out_dram[:], out_sb[:])

    nc.compile()
    return nc


def kernel(Z: np.ndarray, labels: np.ndarray) -> np.ndarray:
    Z = np.asarray(Z, dtype=np.float32)
    labels = np.asarray(labels, dtype=np.int32)

    if "nc" not in _cache:
        _cache["nc"] = _build_program()
    nc = _cache["nc"]

    in_maps = []
    for k in range(NCORES):
        zs = Z[k * PER : (k + 1) * PER]
        ls = labels[k * PER : (k + 1) * PER]
        zp = np.zeros([PAD, D], np.float32)
        zp[:PER] = zs
        lp = np.zeros([PAD], np.int32)
        lp[:PER] = ls
        in_maps.append({"Z": zp, "labels": lp})

    res = run_bass_kernel_spmd(nc, in_maps, core_ids=list(range(NCORES)))
    _cache["last_results"] = res

    gj = np.zeros([C, D, D], np.float64)
    for r in res.results:
        g = r["grams"].astype(np.float64).reshape(4, D, MW).sum(axis=0)
        for j in range(C):
            gj[j] += g[:, j * D : (j + 1) * D]

    g_all = gj.sum(axis=0)
    tr_pi = np.bincount(labels, minlength=C).astype(np.float64)

    nf, df = float(N), float(D)
    eye = np.eye(D)
    loss_r = 0.5 * np.linalg.slogdet(eye + (df / (nf * EPS)) * g_all)[1]
    loss_rc = 0.0
    for j in range(C):
        ld = np.linalg.slogdet(eye + (df / (tr_pi[j] * EPS)) * gj[j])[1]
        loss_rc += (tr_pi[j] / (2.0 * nf)) * ld
    loss_obj = loss_r - loss_rc
    return np.asarray([-loss_obj, loss_r, loss_rc], dtype=np.float32)



# revision 2
# speedup vs baseline: 5.2596x; 5.2596x over previous
"""MCR2 loss kernel for 8 Trainium2 NeuronCores.

Class-sorted data-parallel sharding: the host permutes samples so each
class occupies a contiguous, zero-padded block of CAP rows (one-hot
masking then costs nothing on device).  Each core streams its 76800-row
shard of the permuted Z once as fp16 and accumulates plain per-chunk
Grams Z_chunk^T @ Z_chunk into PSUM; every 7680-row chunk lies entirely
inside one class block, so chunk-Grams sum to class-Grams on the host.
The tiny [10,32,32] partials are reduced on the host and the 32x32
logdets are evaluated there in float64.
"""

import sys

sys.path.insert(0, "/opt/trn_rl_repo")

import numpy as np

import concourse.bacc as bacc
import concourse.bass as bass  # noqa: F401  (kept for parity with bacc deps)
import concourse.mybir as mybir
import concourse.tile as tile
from concourse.bass_utils import run_bass_kernel_spmd

N, D, C = 600000, 32, 10
EPS = 0.5
NCORES = 8
CAP = 61440                      # padded rows per class block (~60000 + 6 sigma)
ROWS_PER_CORE = C * CAP // NCORES        # 76800
CHUNK_TILES = 60                 # 128-sample matmul tiles per chunk
CHUNK_ROWS = 128 * CHUNK_TILES           # 7680
CHUNKS_PER_CORE = ROWS_PER_CORE // CHUNK_ROWS    # 10
CHUNKS_PER_CLASS = CAP // CHUNK_ROWS             # 8

_cache = {}


def _build_program():
    nc = bacc.Bacc(None)
    f16 = mybir.dt.float16
    f32 = mybir.dt.float32
    z_dram = nc.dram_tensor("Z", [ROWS_PER_CORE, D], f16, kind="ExternalInput")
    out_dram = nc.dram_tensor(
        "grams", [128, CHUNKS_PER_CORE * D], f32, kind="ExternalOutput"
    )

    with tile.TileContext(nc) as tc:
        with (
            tc.tile_pool(name="z", bufs=CHUNKS_PER_CORE) as z_pool,
            tc.tile_pool(name="outp", bufs=1) as out_pool,
            tc.tile_pool(name="psum", bufs=1, space="PSUM") as psum_pool,
        ):
            acc = psum_pool.tile([128, CHUNKS_PER_CORE * D], f32)

            zv = z_dram.rearrange("(c p t) d -> c p (t d)", p=128, t=CHUNK_TILES)

            # Issue every chunk DMA up front, alternating between the two
            # hardware DGE queues (Sync and Scalar engines) so both pull
            # from HBM concurrently; the whole shard fits in SBUF.
            z_tiles = []
            for c in range(CHUNKS_PER_CORE):
                z_sb = z_pool.tile([128, CHUNK_TILES * D], f16, tag="z")
                eng = nc.sync if c % 2 == 0 else nc.scalar
                eng.dma_start(z_sb[:], zv[c])
                z_tiles.append(z_sb)

            # Chunk c's Gram accumulates into PSUM columns [c*32, (c+1)*32).
            # Four PE column strips (tile_position) hold four stationaries
            # at once so back-to-back matmuls pipeline; the four 32-row
            # band partials are summed on the host.
            for c in range(CHUNKS_PER_CORE):
                z_sb = z_tiles[c]
                for t in range(CHUNK_TILES):
                    band = t % 4
                    zt = z_sb[:, t * D : (t + 1) * D]
                    nc.tensor.matmul(
                        acc[band * D : (band + 1) * D, c * D : (c + 1) * D],
                        zt,
                        zt,
                        start=(t < 4),
                        stop=(t >= CHUNK_TILES - 4),
                        tile_position=(0, band * D),
                    )

            out_sb = out_pool.tile([128, CHUNKS_PER_CORE * D], f32)
            nc.vector.tensor_copy(out_sb[:], acc[:])
            nc.sync.dma_start(out_dram[:], out_sb[:])

    nc.compile()
    return nc


def kernel(Z: np.ndarray, labels: np.ndarray) -> np.ndarray:
    Z = np.asarray(Z, dtype=np.float32)
    labels = np.asarray(labels, dtype=np.int32)

    if "nc" not in _cache:
        _cache["nc"] = _build_program()
    nc = _cache["nc"]

    counts = np.bincount(labels, minlength=C)
    order = np.argsort(labels, kind="stable")

    Zp = np.zeros([C * CAP, D], np.float16)
    host_extra = np.zeros([C, D, D], np.float64)
    off = 0
    for j in range(C):
        cnt = int(counts[j])
        take = min(cnt, CAP)
        Zp[j * CAP : j * CAP + take] = Z[order[off : off + take]]
        if cnt > CAP:
            extra = Z[order[off + take : off + cnt]].astype(np.float64)
            host_extra[j] = extra.T @ extra
        off += cnt

    in_maps = [
        {"Z": Zp[k * ROWS_PER_CORE : (k + 1) * ROWS_PER_CORE]}
        for k in range(NCORES)
    ]

    res = run_bass_kernel_spmd(nc, in_maps, core_ids=list(range(NCORES)))
    _cache["last_results"] = res

    gj = host_extra.copy()
    for k, r in enumerate(res.results):
        # [128, 320] -> bands summed -> [32, 10, 32] per-chunk partials
        g = r["grams"].astype(np.float64).reshape(4, D, CHUNKS_PER_CORE, D).sum(axis=0)
        for c in range(CHUNKS_PER_CORE):
            gj[(k * CHUNKS_PER_CORE + c) // CHUNKS_PER_CLASS] += g[:, c, :]

    g_all = gj.sum(axis=0)
    tr_pi = counts.astype(np.float64)

    nf, df = float(N), float(D)
    eye = np.eye(D)
    loss_r = 0.5 * np.linalg.slogdet(eye + (df / (nf * EPS)) * g_all)[1]
    loss_rc = 0.0
    for j in range(C):
        ld = np.linalg.slogdet(eye + (df / (tr_pi[j] * EPS)) * gj[j])[1]
        loss_rc += (tr_pi[j] / (2.0 * nf)) * ld
    loss_obj = loss_r - loss_rc
    return np.asarray([-loss_obj, loss_r, loss_rc], dtype=np.float32)


# revision 3
# speedup vs baseline: 6.9927x; 1.3295x over previous
"""MCR2 loss kernel for 8 Trainium2 NeuronCores.

Class-sorted data-parallel sharding: the host permutes samples so each
class occupies a contiguous, zero-padded block of CAP rows (one-hot
masking then costs nothing on device).  Each core streams its 76800-row
shard of the permuted Z once as fp16.  Tensor work is batched four
128-sample tiles per matmul: stationary = moving = a [128, 128] column
block of four adjacent tiles, accumulated into one [128, 128] PSUM
region per 7680-row chunk.  The four diagonal 32x32 blocks of each
region are the per-tile Grams (off-diagonal cross blocks accumulate
into distinct PSUM addresses and are never read).  Every chunk lies
inside one class block, so chunk-Grams sum to class-Grams on the host,
where the 32x32 logdets are evaluated in float64.
"""

import sys

sys.path.insert(0, "/opt/trn_rl_repo")

import numpy as np

import concourse.bacc as bacc
import concourse.bass as bass  # noqa: F401  (kept for parity with bacc deps)
import concourse.mybir as mybir
import concourse.tile as tile
from concourse.bass_utils import run_bass_kernel_spmd

N, D, C = 600000, 32, 10
EPS = 0.5
NCORES = 8
CAP = 61440                      # padded rows per class block (~60000 + 6 sigma)
ROWS_PER_CORE = C * CAP // NCORES        # 76800
CHUNK_TILES = 60                 # 128-sample matmul tiles per chunk
CHUNK_ROWS = 128 * CHUNK_TILES           # 7680
CHUNKS_PER_CORE = ROWS_PER_CORE // CHUNK_ROWS    # 10
CHUNKS_PER_CLASS = CAP // CHUNK_ROWS             # 8
GROUPS = CHUNK_TILES // 4        # 15 four-tile matmul groups per chunk
SPLIT = 32                       # tiles 0..31 arrive on the sync queue, rest on scalar

_cache = {}


def _build_program():
    nc = bacc.Bacc(None)
    f16 = mybir.dt.float16
    f32 = mybir.dt.float32
    z_dram = nc.dram_tensor("Z", [ROWS_PER_CORE, D], f16, kind="ExternalInput")
    out_dram = nc.dram_tensor(
        "grams", [128, CHUNKS_PER_CORE * D], f32, kind="ExternalOutput"
    )

    with tile.TileContext(nc) as tc:
        with (
            tc.tile_pool(name="z", bufs=CHUNKS_PER_CORE) as z_pool,
            tc.tile_pool(name="outp", bufs=1) as out_pool,
            tc.tile_pool(name="psum", bufs=1, space="PSUM") as psum_pool,
        ):
            acc = psum_pool.tile([128, CHUNKS_PER_CORE * 128], f32)

            zv = z_dram.rearrange("(c p t) d -> c p (t d)", p=128, t=CHUNK_TILES)

            # Every chunk DMA is issued up front, split between the two
            # hardware DGE queues (Sync and Scalar engines) so both pull
            # from HBM at once; the whole shard stays resident in SBUF.
            z_tiles = []
            for c in range(CHUNKS_PER_CORE):
                z_sb = z_pool.tile([128, CHUNK_TILES * D], f16, tag="z")
                nc.sync.dma_start(z_sb[:, : SPLIT * D], zv[c][:, : SPLIT * D])
                nc.scalar.dma_start(z_sb[:, SPLIT * D :], zv[c][:, SPLIT * D :])
                z_tiles.append(z_sb)

            for c in range(CHUNKS_PER_CORE):
                z_sb = z_tiles[c]
                for g in range(GROUPS):
                    zg = z_sb[:, g * 4 * D : (g + 1) * 4 * D]
                    nc.tensor.matmul(
                        acc[:, c * 128 : (c + 1) * 128],
                        zg,
                        zg,
                        start=(g == 0),
                        stop=(g == GROUPS - 1),
                    )

            # Pull the four diagonal 32x32 blocks of every chunk region out
            # to SBUF, two bands on the DVE and two on the Activation engine.
            out_sb = out_pool.tile([128, CHUNKS_PER_CORE * D], f32)
            acc3 = acc[:].rearrange("p (c x) -> p c x", x=128)
            out3 = out_sb[:].rearrange("p (c x) -> p c x", x=D)
            for b in range(4):
                src = acc3[b * D : (b + 1) * D, :, b * D : (b + 1) * D]
                dst = out3[b * D : (b + 1) * D, :, :]
                if b < 2:
                    nc.vector.tensor_copy(dst, src)
                else:
                    nc.scalar.mul(dst, src, 1.0)
            nc.sync.dma_start(out_dram[:], out_sb[:])

    nc.compile()
    return nc


def kernel(Z: np.ndarray, labels: np.ndarray) -> np.ndarray:
    Z = np.asarray(Z, dtype=np.float32)
    labels = np.asarray(labels, dtype=np.int32)

    if "nc" not in _cache:
        _cache["nc"] = _build_program()
    nc = _cache["nc"]

    counts = np.bincount(labels, minlength=C)
    order = np.argsort(labels, kind="stable")

    Zp = np.zeros([C * CAP, D], np.float16)
    host_extra = np.zeros([C, D, D], np.float64)
    off = 0
    for j in range(C):
        cnt = int(counts[j])
        take = min(cnt, CAP)
        Zp[j * CAP : j * CAP + take] = Z[order[off : off + take]]
        if cnt > CAP:
            extra = Z[order[off + take : off + cnt]].astype(np.float64)
            host_extra[j] = extra.T @ extra
        off += cnt

    in_maps = [
        {"Z": Zp[k * ROWS_PER_CORE : (k + 1) * ROWS_PER_CORE]}
        for k in range(NCORES)
    ]

    res = run_bass_kernel_spmd(nc, in_maps, core_ids=list(range(NCORES)))
    _cache["last_results"] = res

    gj = host_extra.copy()
    for k, r in enumerate(res.results):
        # [128, 320] -> bands summed -> [32, 10, 32] per-chunk partials
        g = r["grams"].astype(np.float64).reshape(4, D, CHUNKS_PER_CORE, D).sum(axis=0)
        for c in range(CHUNKS_PER_CORE):
            gj[(k * CHUNKS_PER_CORE + c) // CHUNKS_PER_CLASS] += g[:, c, :]

    g_all = gj.sum(axis=0)
    tr_pi = counts.astype(np.float64)

    nf, df = float(N), float(D)
    eye = np.eye(D)
    loss_r = 0.5 * np.linalg.slogdet(eye + (df / (nf * EPS)) * g_all)[1]
    loss_rc = 0.0
    for j in range(C):
        ld = np.linalg.slogdet(eye + (df / (tr_pi[j] * EPS)) * gj[j])[1]
        loss_rc += (tr_pi[j] / (2.0 * nf)) * ld
    loss_obj = loss_r - loss_rc
    return np.asarray([-loss_obj, loss_r, loss_rc], dtype=np.float32)
